# revision 12
# baseline (speedup 1.0000x reference)
"""Self-contained Trainium2 Bass kernel for the 2-layer GAT (nn_GAT_6451040878848).

Sharding: nodes are permuted by quantized in-degree and dealt round-robin to 8
cores; core k owns a contiguous 12800-row octant of the permuted node table
(tile 64 of each octant is a reserved all-pad tile) and aggregates every edge
whose dst is in its octant (dst-octant edge sharding -> no all-reduce). The
only collective is one bf16 AllGather of the layer-2 node table.

Edge phase: per 128-dst tile, edges are slot-major (slot c = c-th in-edge of
each dst; dst == partition), gathered from the node table with dma_gather
(int16 idx). The table is addressed through THREE overlapping 65536-row bank
windows (A=[0,64K), B=[18432,83968), C=[36864,102400)); edges whose src row
falls in an overlap are routed to whichever bank balances the per-dst slot
counts (cuts pad slots ~29% vs a 2-bank split). Pad slots point at zero rows
(alpha_s=-40 => w~e^-8) inside bank-addressable pad tiles, so every index is
non-negative. Weights w = exp(leaky_relu(alpha_s[src]+alpha_d[dst])) (no
max-subtraction: logits are O(1)); the scatter is an identity-lhsT matmul
accumulating 4 slots per Matmult into a [128, 4*vw] PSUM tile, folded with one
tensor_reduce; the softmax denominator rides along as extra rhs columns.
alpha_d never round-trips through DRAM: layer-1 alpha_d is copied to SBUF
during the (replicated) node phase, layer-2 alpha_d is written to SBUF by the
octant-local layer-2 node phase.
"""
import numpy as np
import ml_dtypes

import concourse.bacc as bacc
import concourse.bass as bass
import concourse.tile as tile
from concourse import mybir
from concourse.bass_utils import run_bass_kernel_spmd

P = 128
TROW = 128           # bf16 elements per node-table row (256B)
NEG_SLOPE = 0.2
F_IN = 128
H1, C1 = 8, 8
C2 = 64
DEG_QUANT = 4
NCORES = 8
BANK = 65536
MMG = 4              # slots per matmul group

# ---- fixed geometry (N = 100000 hardcoded) ----
N_NODES = 100000
PER_CORE = 12800             # 100 tiles; tile PAD_TILE is all-pad
N_TILES = PER_CORE // P      # 100
PAD_TILE = 64
J_REAL = 12544               # dealt node slots per core (before pad-tile insertion)
NPAD = PER_CORE * NCORES     # 102400 == rows_total
BASE_A = 32768
B_START = 18432
C_START = NPAD - BANK        # 36864
BASE_B = B_START + 32768     # 51200
BASE_C = C_START + 32768     # 69632
ROW_PAD_A = 4 * PER_CORE + PAD_TILE * P   # 59392 (octant 4 pad tile)
ROW_PAD_C = 5 * PER_CORE + PAD_TILE * P   # 72192 (octant 5 pad tile)
ROW_PAD_B = ROW_PAD_A                      # also in bank B's positive window

bf16 = ml_dtypes.bfloat16


# ----------------------------------------------------------------------------
# Host-side graph preprocessing (integer/index work only)
# ----------------------------------------------------------------------------
def host_prep(edge_index: np.ndarray, n_nodes: int, n_cores: int = NCORES):
    assert n_nodes == N_NODES and n_cores == NCORES
    N = n_nodes
    loops = np.arange(N, dtype=np.int64)
    src = np.concatenate([edge_index[0].astype(np.int64), loops])
    dst = np.concatenate([edge_index[1].astype(np.int64), loops])

    deg = np.bincount(dst, minlength=N)
    degq = -(-deg // DEG_QUANT) * DEG_QUANT

    rank = np.argsort(degq, kind="stable")
    node_rank = np.empty(N, np.int64)
    node_rank[rank] = np.arange(N)
    core_of = node_rank % n_cores
    j_of = node_rank // n_cores              # < 12500 <= J_REAL
    l_of = j_of + np.where(j_of >= PAD_TILE * P, P, 0)
    pos = core_of * PER_CORE + l_of          # == table row
    node_of_pos = np.full(NPAD, -1, dtype=np.int64)
    node_of_pos[pos] = np.arange(N)

    e_core = core_of[dst]
    e_tile = l_of[dst] // P
    e_part = l_of[dst] % P
    r = pos[src]

    inA = r < BANK
    inB = (r >= B_START) & (r < B_START + BANK)
    inC = r >= C_START
    # class 0: A only, 1: A|B, 2: A|B|C, 3: B|C, 4: C only
    cls = np.where(inA & ~inB, 0,
          np.where(inA & inB & ~inC, 1,
          np.where(inA & inB & inC, 2,
          np.where(inB & ~inA, 3, 4))))

    cnt = np.zeros((5, n_cores, N_TILES, P), np.int32)
    for c in range(5):
        m = cls == c
        np.add.at(cnt[c], (e_core[m], e_tile[m], e_part[m]), 1)

    # ---- per-tile (DA, DB, DC) optimization ----
    DA = np.zeros(N_TILES, np.int64)
    DB = np.zeros(N_TILES, np.int64)
    DC = np.zeros(N_TILES, np.int64)
    SPAN = 14
    for t in range(N_TILES):
        n1 = cnt[0, :, t, :].ravel().astype(np.int64)
        n2 = cnt[1, :, t, :].ravel().astype(np.int64)
        n3 = cnt[2, :, t, :].ravel().astype(np.int64)
        n4 = cnt[3, :, t, :].ravel().astype(np.int64)
        n5 = cnt[4, :, t, :].ravel().astype(np.int64)
        if (n1 + n2 + n3 + n4 + n5).max() == 0:
            continue
        da0 = int(n1.max()); dc0 = int(n5.max())
        das = np.arange(da0, da0 + SPAN)[:, None, None]       # [SPAN,1,1]
        dcs = np.arange(dc0, dc0 + SPAN)[None, :, None]       # [1,SPAN,1]
        spareA = das - n1[None, None, :]
        spareC = dcs - n5[None, None, :]
        ov2 = np.maximum(0, n2[None, None, :] - spareA)
        ov4 = np.maximum(0, n4[None, None, :] - spareC)
        sA2 = np.maximum(0, spareA - n2[None, None, :])
        sC2 = np.maximum(0, spareC - n4[None, None, :])
        ov3 = np.maximum(0, n3[None, None, :] - sA2 - sC2)
        db = (ov2 + ov4 + ov3).max(axis=2)                    # [SPAN,SPAN]
        tot = das[:, :, 0] + dcs[:, :, 0] + db
        i, j = np.unravel_index(np.argmin(tot), tot.shape)
        DA[t] = da0 + i; DC[t] = dc0 + j; DB[t] = int(db[i, j])

    # ---- per-dst routing (greedy, honoring (DA, DB, DC)) ----
    # bank_of_edge: 0=A, 1=B, 2=C
    bank = np.full(len(src), -1, dtype=np.int8)
    dstkey = (e_core * N_TILES + e_tile) * P + e_part
    order = np.lexsort((cls, dstkey))
    sk = dstkey[order]; sc = cls[order]
    grp_start = np.concatenate([[0], np.nonzero(np.diff(sk))[0] + 1, [len(sk)]])
    ebank = np.empty(len(order), np.int8)
    slot = np.empty(len(order), np.int64)
    tile_of_key = (np.arange(n_cores * N_TILES * P) // P) % N_TILES
    for gi in range(len(grp_start) - 1):
        s0, s1 = grp_start[gi], grp_start[gi + 1]
        key = sk[s0]; t = tile_of_key[key]
        da, db, dc = int(DA[t]), int(DB[t]), int(DC[t])
        cl = sc[s0:s1]
        c0 = int((cl == 0).sum()); c1 = int((cl == 1).sum())
        c2 = int((cl == 2).sum()); c3 = int((cl == 3).sum())
        c4 = int((cl == 4).sum())
        aspare = da - c0; cspare = dc - c4
        q1A = min(c1, aspare)
        q3C = min(c3, cspare)
        aspare2 = aspare - q1A; cspare2 = cspare - q3C
        q2A = min(c2, aspare2)
        q2C = min(c2 - q2A, cspare2)
        nB = (c1 - q1A) + (c3 - q3C) + (c2 - q2A - q2C)
        assert aspare >= 0 and cspare >= 0 and nB <= db, (t, da, db, dc)
        bk = np.empty(s1 - s0, np.int8)
        u1 = u2 = u3 = 0
        for i, c in enumerate(cl):
            if c == 0:
                bk[i] = 0
            elif c == 4:
                bk[i] = 2
            elif c == 1:
                bk[i] = 0 if u1 < q1A else 1
                u1 += 1
            elif c == 3:
                bk[i] = 2 if u3 < q3C else 1
                u3 += 1
            else:
                bk[i] = 0 if u2 < q2A else (2 if u2 < q2A + q2C else 1)
                u2 += 1
        # segment-relative slots, in class order of appearance
        sl = np.empty(s1 - s0, np.int64)
        ca = cb = cc = 0
        for i in range(s1 - s0):
            if bk[i] == 0:
                sl[i] = ca; ca += 1
            elif bk[i] == 1:
                sl[i] = cb; cb += 1
            else:
                sl[i] = cc; cc += 1
        ebank[s0:s1] = bk
        slot[s0:s1] = sl

    # ---- trailing-negative terminators ----
    # The ucode drops a contiguous all-negative tail of each gather's idx
    # list, so the LAST list position (slot d-1, part 127) of each bank
    # segment must be non-negative. If partition 127's segment is full of
    # negative-idx reals for some core, widen that bank by one all-pad slot.
    bases = np.array([BASE_A, BASE_B, BASE_C], np.int64)
    padrow = np.array([ROW_PAD_A, ROW_PAD_B, ROW_PAD_C], np.int64)
    so_core = e_core[order]; so_row = r[order]
    so_tile = e_tile[order]; so_part = e_part[order]
    so_bank = ebank; so_slot = slot
    Ds = np.stack([DA, DB, DC], axis=1)           # [T, 3]
    m127 = so_part == 127
    for t in range(N_TILES):
        for b in range(3):
            d = int(Ds[t, b])
            if d == 0:
                continue
            m = m127 & (so_tile == t) & (so_bank == b)
            if not m.any():
                continue
            biased = so_row[m] - bases[b]
            cores = so_core[m]
            for k in np.unique(cores):
                mk = cores == k
                if mk.sum() == d and (biased[mk] < 0).all():
                    Ds[t, b] = d + 1
                    break
    DA, DB, DC = Ds[:, 0], Ds[:, 1], Ds[:, 2]

    # ---- idx16 arrays ----
    Dsum = DA + DB + DC
    col_off = np.concatenate([[0], np.cumsum(Dsum * 8)]).astype(np.int64)
    idx_cols = int(col_off[-1])
    idx16 = np.zeros((n_cores, 16, max(idx_cols, 1)), np.int16)
    for t in range(N_TILES):
        ds = int(Dsum[t])
        if ds == 0:
            continue
        tab = np.empty((n_cores, ds, P), np.int64)
        for b in range(3):
            lo = (0, DA[t], DA[t] + DB[t])[b]
            d = (DA[t], DB[t], DC[t])[b]
            if d:
                tab[:, lo:lo + d, :] = padrow[b] - bases[b]
        m = so_tile == t
        seg_lo = np.array([0, DA[t], DA[t] + DB[t]], np.int64)
        abs_slot = seg_lo[so_bank[m]] + so_slot[m]
        biased = so_row[m] - bases[so_bank[m]]
        tab[so_core[m], abs_slot, so_part[m]] = biased
        assert tab.min() >= -32768 and tab.max() <= 32767
        for b in range(3):
            lo = (0, DA[t], DA[t] + DB[t])[b]
            d = int((DA[t], DB[t], DC[t])[b])
            if d == 0:
                continue
            for k in range(n_cores):
                col = tab[k, lo:lo + d, 127]
                if col[d - 1] < 0:
                    nn = np.nonzero(col >= 0)[0]
                    assert len(nn), (t, b, k, "all-negative p127 segment")
                    j = nn[-1]
                    col[d - 1], col[j] = col[j], col[d - 1]
        lo_c = int(col_off[t])
        idx16[:, :, lo_c:lo_c + ds * 8] = (
            tab.reshape(n_cores, -1, 16).transpose(0, 2, 1).astype(np.int16))
    idx16 = np.tile(idx16, (1, 8, 1))

    meta = dict(
        N=N, Npad=NPAD, n_cores=n_cores, per_core=PER_CORE, n_tiles=N_TILES,
        DA=[int(v) for v in DA], DB=[int(v) for v in DB], DC=[int(v) for v in DC],
        col_off=[int(v) for v in col_off], idx_cols=int(max(idx_cols, 1)),
    )
    return meta, idx16, node_of_pos


# ----------------------------------------------------------------------------
# Device kernel
# ----------------------------------------------------------------------------
def build_kernel(meta):
    import os
    Npad = meta["Npad"]; n_cores = meta["n_cores"]; per_core = meta["per_core"]
    n_tiles = meta["n_tiles"]
    DA, DB, DC = meta["DA"], meta["DB"], meta["DC"]
    col_off = meta["col_off"]; idx_cols = meta["idx_cols"]
    n_groups_per_oct = per_core // (4 * P)        # 25

    nc = bacc.Bacc("TRN2", target_bir_lowering=False, debug=False,
                   num_devices=n_cores, num_swdge_queues=4)
    f32, b16, i16 = mybir.dt.float32, mybir.dt.bfloat16, mybir.dt.int16
    AF = mybir.ActivationFunctionType
    OP = mybir.AluOpType

    xT = nc.dram_tensor("xT", [F_IN, Npad], b16, kind="ExternalInput").ap()
    W1 = nc.dram_tensor("W1", [F_IN, 64], f32, kind="ExternalInput").ap()
    W1T = nc.dram_tensor("W1T", [64, F_IN], f32, kind="ExternalInput").ap()
    A1 = nc.dram_tensor("A1", [64, 16], f32, kind="ExternalInput").ap()
    W2 = nc.dram_tensor("W2", [64, C2], f32, kind="ExternalInput").ap()
    W2T = nc.dram_tensor("W2T", [C2, 64], f32, kind="ExternalInput").ap()
    A2 = nc.dram_tensor("A2", [C2, 2], f32, kind="ExternalInput").ap()
    B1 = nc.dram_tensor("B1", [1, 64], f32, kind="ExternalInput").ap()
    B2 = nc.dram_tensor("B2", [1, C2], f32, kind="ExternalInput").ap()
    IDX = nc.dram_tensor("IDX", [P, idx_cols], i16, kind="ExternalInput").ap()
    OUT = nc.dram_tensor("OUT", [per_core, C2], f32, kind="ExternalOutput").ap()

    with tile.TileContext(nc) as tc:
        with tc.tile_pool(name="dram", bufs=1, space="DRAM") as dram, \
             tc.tile_pool(name="consts", bufs=1) as cp, \
             tc.tile_pool(name="stg", bufs=3) as nsp, \
             tc.tile_pool(name="xtp", bufs=3) as xtp, \
             tc.tile_pool(name="gpl", bufs=3) as gp, \
             tc.tile_pool(name="idxp", bufs=3) as idxp, \
             tc.tile_pool(name="vwp", bufs=3) as vwp, \
             tc.tile_pool(name="stat", bufs=4) as sp, \
             tc.tile_pool(name="pacc", bufs=2, space="PSUM") as pacc, \
             tc.tile_pool(name="pnode", bufs=2, space="PSUM") as pnode, \
             tc.tile_pool(name="ptr", bufs=2, space="PSUM") as ptr, \
             tc.tile_pool(name="pl2", bufs=2, space="PSUM") as pl2:

            table1 = dram.tile([Npad, TROW], b16)
            table2 = dram.tile([Npad, TROW], b16)
            h2loc = dram.tile([per_core, TROW], b16)

            # ------------- constants -------------
            ident = cp.tile([P, P], b16)
            nc.gpsimd.memset(ident[:], 0.0)
            iota_i = cp.tile([P, 1], mybir.dt.int32)
            nc.gpsimd.iota(iota_i[:], pattern=[[0, 1]], base=0, channel_multiplier=1)
            iota_f = cp.tile([P, 1], f32)
            nc.vector.tensor_copy(out=iota_f[:], in_=iota_i[:])
            iotar_i = cp.tile([P, P], mybir.dt.int32)
            nc.gpsimd.iota(iotar_i[:], pattern=[[1, P]], base=0, channel_multiplier=0)
            iotar_f = cp.tile([P, P], f32)
            nc.vector.tensor_copy(out=iotar_f[:], in_=iotar_i[:])
            nc.vector.tensor_scalar(out=ident[:], in0=iotar_f[:], scalar1=iota_f[:],
                                    scalar2=None, op0=OP.is_equal)

            w1f = cp.tile([P, 64], f32)
            nc.sync.dma_start(out=w1f[:], in_=W1)
            w1t = cp.tile([64, P], f32)
            nc.sync.dma_start(out=w1t[:], in_=W1T)
            a1t = cp.tile([64, 16], f32)
            nc.sync.dma_start(out=a1t[:], in_=A1)
            w2f = cp.tile([64, 64], f32)
            nc.sync.dma_start(out=w2f[:], in_=W2)
            w2t = cp.tile([64, 64], f32)
            nc.sync.dma_start(out=w2t[:], in_=W2T)
            a2t = cp.tile([64, 2], f32)
            nc.sync.dma_start(out=a2t[:], in_=A2)
            b1r = cp.tile([1, 64], f32)
            nc.sync.dma_start(out=b1r[:], in_=B1)
            b1b = cp.tile([P, 64], f32)
            nc.gpsimd.partition_broadcast(b1b[:], b1r[:])
            b2r = cp.tile([1, 64], f32)
            nc.sync.dma_start(out=b2r[:], in_=B2)
            b2b = cp.tile([P, 64], f32)
            nc.gpsimd.partition_broadcast(b2b[:], b2r[:])

            wext1 = cp.tile([P, 80], b16)
            ws_ps = pnode.tile([P, 352], f32, space="PSUM", tag="np")
            nc.tensor.matmul(out=ws_ps[:, 0:16], lhsT=w1t[:], rhs=a1t[:], start=True, stop=True)
            nc.vector.tensor_copy(out=wext1[:, 0:64], in_=w1f[:])
            nc.vector.tensor_copy(out=wext1[:, 64:80], in_=ws_ps[:, 0:16])

            w2ext = cp.tile([64, 66], b16)
            ws2_ps = pnode.tile([P, 352], f32, space="PSUM", tag="np")
            nc.tensor.matmul(out=ws2_ps[:64, 0:2], lhsT=w2t[:], rhs=a2t[:], start=True, stop=True)
            nc.vector.tensor_copy(out=w2ext[:, 0:64], in_=w2f[:])
            nc.vector.tensor_copy(out=w2ext[:, 64:66], in_=ws2_ps[:64, 0:2])

            # pad-row template: h = 0, alpha = -40
            padt = cp.tile([P, 16], b16)
            nc.gpsimd.memset(padt[:], -40.0)
            padt2 = cp.tile([P, 66], b16)
            nc.gpsimd.memset(padt2[:], 0.0)
            nc.gpsimd.memset(padt2[:, 64:66], -40.0)

            # alpha_d SBUF tables
            ad1_all = cp.tile([P, n_cores * n_tiles * 8], b16)
            ad2_all = cp.tile([P, n_tiles], b16)

            pid = nc.partition_id()

            # ------------- L1 node phase (replicated) -------------
            # Packed layout: matmul i uses lhsT = xt[:, i::4] so output
            # partition p' holds node 4p'+i; the 512-row table store is then
            # per-partition contiguous (128 descs x 1KB instead of 512 x 160B).
            for k in range(n_cores):
                for g in range(n_groups_per_oct):
                    base = k * per_core + g * 4 * P
                    xt = xtp.tile([P, 4 * P], b16, tag="xt")
                    nc.sync.dma_start(out=xt[:], in_=xT[:, base:base + 4 * P])
                    xtv = xt[:].rearrange("f (s i) -> f s i", i=4)
                    ps = pnode.tile([P, 352], f32, space="PSUM", tag="np")
                    stage = nsp.tile([P, 4 * TROW], b16, tag="stage")
                    for i in range(4):
                        nc.tensor.matmul(out=ps[:, i * 72:(i + 1) * 72],
                                         lhsT=xtv[:, :, i],
                                         rhs=wext1[:, 0:72], start=True, stop=True)
                    # alpha_d (node-per-partition) in cols 288:320
                    for s in range(4):
                        nc.tensor.matmul(out=ps[:, 288 + s * 8:288 + (s + 1) * 8],
                                         lhsT=xt[:, s * P:(s + 1) * P],
                                         rhs=wext1[:, 72:80], start=True, stop=True)
                    nc.scalar.activation(
                        out=stage[:].rearrange("p (i r) -> p i r", r=TROW)[:, :, 0:72],
                        in_=ps[:, 0:288].rearrange("p (i r) -> p i r", r=72),
                        func=AF.Copy)
                    nc.sync.dma_start(
                        out=table1[base:base + 4 * P, :].rearrange("(p i) r -> p (i r)", p=P),
                        in_=stage[:])
                    nc.vector.tensor_copy(
                        out=ad1_all[:, (k * n_tiles + g * 4) * 8:(k * n_tiles + g * 4 + 4) * 8],
                        in_=ps[:, 288:320])
            # patch pad tiles: alpha_s (cols 64:72) = -40  (h already 0)
            for k in range(n_cores):
                r0 = k * per_core + PAD_TILE * P
                nc.sync.dma_start(
                    out=table1[r0:r0 + P, 64:72].rearrange("(o p) r -> p (o r)", p=P),
                    in_=padt[:, 0:8])

            # ------------- edge phase -------------
            def edge_phase(table, layer, emit):
                heads = H1 if layer == 1 else 1
                vw = 72 if layer == 1 else 65
                in_ap = (table[BASE_A:BASE_A + 32768, :],
                         table[BASE_B:BASE_B + 32768, :],
                         table[BASE_C:min(BASE_C + 32768, Npad), :])
                qctr = 0
                for t in range(n_tiles):
                    da, db, dc = DA[t], DB[t], DC[t]
                    ds = da + db + dc
                    if ds == 0:
                        continue
                    lo_c = col_off[t]
                    idxt = idxp.tile([P, ds * 8], i16, tag="idx")
                    nc.sync.dma_start(out=idxt[:], in_=IDX[:, lo_c:lo_c + ds * 8])
                    G = gp.tile([P, ds * TROW], b16, tag="G")
                    for b, (off, d) in enumerate(((0, da), (da, db), (da + db, dc))):
                        if d == 0:
                            continue
                        nc.gpsimd.dma_gather(
                            out_ap=G[:, off * TROW:(off + d) * TROW]
                                .rearrange("p (s r) -> p s r", r=TROW),
                            in_ap=in_ap[b], idxs_ap=idxt[:, off * 8:(off + d) * 8],
                            num_idxs=d * P, num_idxs_reg=d * P, elem_size=TROW,
                            queue_num=qctr % 4, single_packet=False)
                        qctr += 1

                    Gv = G[:].rearrange("p (s r) -> p s r", r=TROW)
                    if layer == 1:
                        ad_off = pid * (n_tiles * 8) + t * 8
                        ad_t = sp.tile([P, 8], b16, tag="adt")
                        nc.vector.tensor_copy(out=ad_t[:], in_=ad1_all[:, bass.ds(ad_off, 8)])
                    w_all = sp.tile([P, ds * heads], f32, tag="wf")
                    wb_all = sp.tile([P, ds * heads], b16, tag="wb")
                    e_t = sp.tile([P, ds * heads], f32, tag="et")
                    if layer == 1:
                        adv = ad_t[:].unsqueeze(1).broadcast_to([P, ds, 8])
                        nc.vector.tensor_tensor(
                            out=e_t[:].rearrange("p (s h) -> p s h", h=8),
                            in0=Gv[:, :, 64:72], in1=adv, op=OP.add)
                        nc.scalar.activation(out=e_t[:], in_=e_t[:],
                                             func=AF.Prelu, alpha=NEG_SLOPE)
                    else:
                        nc.scalar.activation(
                            out=e_t[:],
                            in_=Gv[:, :, 64:65].rearrange("p s one -> p (s one)"),
                            func=AF.Prelu, bias=ad2_all[:, t:t + 1], alpha=NEG_SLOPE)
                    nc.scalar.activation(out=w_all[:], in_=e_t[:], func=AF.Exp)
                    nc.vector.tensor_copy(out=wb_all[:], in_=w_all[:])

                    nmm = -(-ds // MMG)
                    ds_pad = nmm * MMG
                    Vw = vwp.tile([P, ds_pad * vw], b16, tag="vw")
                    if ds_pad > ds:
                        nc.gpsimd.memset(Vw[:, ds * vw:ds_pad * vw], 0.0)
                    Vv = Vw[:].rearrange("p (s c) -> p s c", c=vw)[:, 0:ds, :]
                    if layer == 1:
                        wbv = wb_all[:].rearrange("p (s h) -> p s h", h=8) \
                            .unsqueeze(3).broadcast_to([P, ds, 8, 8])
                        nc.vector.tensor_tensor(
                            out=Vv[:, :, 0:64].rearrange("p s (h c) -> p s h c", c=8),
                            in0=Gv[:, :, 0:64].rearrange("p s (h c) -> p s h c", c=8),
                            in1=wbv, op=OP.mult)
                        nc.vector.tensor_copy(
                            out=Vv[:, :, 64:72],
                            in_=wb_all[:].rearrange("p (s h) -> p s h", h=8))
                    else:
                        wbv = wb_all[:].unsqueeze(2).broadcast_to([P, ds, 64])
                        nc.vector.tensor_tensor(
                            out=Vv[:, :, 0:64], in0=Gv[:, :, 0:64], in1=wbv, op=OP.mult)
                        nc.vector.tensor_copy(
                            out=Vv[:, :, 64:65], in_=wb_all[:].unsqueeze(2))

                    acc = pacc.tile([P, MMG * 72], f32, space="PSUM", tag="acc")
                    for i in range(nmm):
                        nc.tensor.matmul(out=acc[:, 0:MMG * vw], lhsT=ident[:],
                                         rhs=Vw[:, i * MMG * vw:(i + 1) * MMG * vw],
                                         start=(i == 0), stop=(i == nmm - 1))
                    emit(t, acc, MMG, vw)

            # ------------- L1 -------------
            out1 = cp.tile([P, n_tiles * 72], b16)

            def emit1(t, acc, ngrp, vw):
                with nc.allow_low_precision(reason="4-way fold to bf16 staging"):
                    nc.vector.tensor_reduce(
                        out=out1[:, t * 72:(t + 1) * 72],
                        in_=acc[:, 0:ngrp * vw].rearrange("p (g c) -> p c g", c=vw),
                        op=OP.add, axis=mybir.AxisListType.X)

            if os.environ.get("SKIP_E1") == "1":
                nc.gpsimd.memset(out1[:], 1.0)
            else:
                edge_phase(table1, 1, emit1)

            # ------------- L2 node phase (octant-local) -------------
            for t in range(n_tiles):
                if t == PAD_TILE:
                    nc.sync.dma_start(out=h2loc[t * P:(t + 1) * P, 0:66], in_=padt2[:])
                    continue
                if DA[t] + DB[t] + DC[t] == 0:
                    continue
                den = sp.tile([P, 8], f32, tag="den")
                nc.vector.tensor_copy(out=den[:], in_=out1[:, t * 72 + 64:t * 72 + 72])
                rec = sp.tile([P, 8], f32, tag="rec")
                nc.vector.reciprocal(rec[:], den[:])
                recb = sp.tile([P, 8], b16, tag="recb")
                nc.vector.tensor_copy(out=recb[:], in_=rec[:])
                h1f = sp.tile([P, 64], f32, tag="h1f")
                nc.vector.tensor_tensor(
                    out=h1f[:].rearrange("p (h c) -> p h c", c=8),
                    in0=out1[:, t * 72:t * 72 + 64].rearrange("p (h c) -> p h c", c=8),
                    in1=recb[:].unsqueeze(2).broadcast_to([P, 8, 8]), op=OP.mult)
                nc.vector.tensor_tensor(out=h1f[:], in0=h1f[:], in1=b1b[:], op=OP.add)
                # ELU: out = max(x,0) + exp(min(x,0)) - 1
                xm = sp.tile([P, 64], f32, tag="xm")
                nc.vector.tensor_scalar(out=xm[:], in0=h1f[:], scalar1=0.0,
                                        scalar2=None, op0=OP.min)
                xe = sp.tile([P, 64], f32, tag="xe")
                nc.scalar.activation(out=xe[:], in_=xm[:], func=AF.Exp)
                xp = sp.tile([P, 64], b16, tag="xp")
                nc.vector.tensor_scalar(out=xp[:], in0=h1f[:], scalar1=0.0,
                                        scalar2=None, op0=OP.max)
                h1e = sp.tile([P, 64], b16, tag="h1e")
                nc.vector.tensor_scalar(out=h1e[:], in0=xe[:], scalar1=-1.0,
                                        scalar2=None, op0=OP.add, accum_out=None)
                nc.vector.tensor_tensor(out=h1e[:], in0=h1e[:], in1=xp[:], op=OP.add)
                trp = ptr.tile([P, P], b16, space="PSUM", tag="tr")
                nc.tensor.transpose(out=trp[:64, :], in_=h1e[:], identity=ident[:])
                h1t = sp.tile([64, P], b16, tag="h1t")
                nc.scalar.activation(out=h1t[:], in_=trp[:64, :], func=AF.Copy)
                ps2 = pl2.tile([P, 80], f32, space="PSUM", tag="l2")
                nc.tensor.matmul(out=ps2[:, 0:66], lhsT=h1t[:], rhs=w2ext[:],
                                 start=True, stop=True)
                st2 = nsp.tile([P, 65], b16, tag="st2")
                nc.scalar.activation(out=st2[:], in_=ps2[:, 0:65], func=AF.Copy)
                nc.sync.dma_start(out=h2loc[t * P:(t + 1) * P, 0:65], in_=st2[:])
                nc.vector.tensor_copy(out=ad2_all[:, t:t + 1], in_=ps2[:, 65:66])

            # ------------- AllGather h2 octants -> table2 -------------
            if os.environ.get("SKIP_CC") == "1":
                for k in range(n_cores):
                    nc.sync.dma_start(
                        out=table2[k * per_core:(k + 1) * per_core, :],
                        in_=h2loc[:])
            else:
                nc.gpsimd.collective_compute(
                    "AllGather", mybir.AluOpType.bypass,
                    replica_groups=[list(range(n_cores))],
                    ins=[h2loc[:].opt()],
                    outs=[table2[0:Npad, :].opt()],
                )

            # ------------- L2 -------------
            def emit2(t, acc, ngrp, vw):
                o2 = sp.tile([P, 65], f32, tag="o2")
                nc.vector.tensor_reduce(
                    out=o2[:],
                    in_=acc[:, 0:ngrp * vw].rearrange("p (g c) -> p c g", c=vw),
                    op=OP.add, axis=mybir.AxisListType.X)
                rec2 = sp.tile([P, 1], f32, tag="rec2")
                nc.vector.reciprocal(rec2[:], o2[:, 64:65])
                o2n = sp.tile([P, 64], f32, tag="o2n")
                nc.vector.tensor_scalar(out=o2n[:], in0=o2[:, 0:64], scalar1=rec2[:],
                                        scalar2=None, op0=OP.mult)
                nc.vector.tensor_tensor(out=o2n[:], in0=o2n[:], in1=b2b[:], op=OP.add)
                m = sp.tile([P, 1], f32, tag="m")
                nc.vector.tensor_reduce(out=m[:], in_=o2n[:], op=OP.max,
                                        axis=mybir.AxisListType.X)
                negm = sp.tile([P, 1], f32, tag="negm")
                nc.vector.tensor_scalar(out=negm[:], in0=m[:], scalar1=-1.0,
                                        scalar2=None, op0=OP.mult)
                scr = sp.tile([P, 64], f32, tag="scr")
                sume = sp.tile([P, 1], f32, tag="sume")
                nc.scalar.activation(out=scr[:], in_=o2n[:], func=AF.Exp,
                                     bias=negm[:], accum_out=sume[:])
                lns = sp.tile([P, 1], f32, tag="lns")
                nc.scalar.activation(out=lns[:], in_=sume[:], func=AF.Ln)
                res = sp.tile([P, 64], f32, tag="res")
                nc.vector.tensor_scalar(out=res[:], in0=o2n[:], scalar1=m[:],
                                        scalar2=lns[:], op0=OP.subtract,
                                        op1=OP.subtract)
                nc.sync.dma_start(out=OUT[t * P:(t + 1) * P, :], in_=res[:])

            if os.environ.get("SKIP_E2") == "1":
                zres = sp.tile([P, 64], f32, tag="zres")
                nc.gpsimd.memset(zres[:], 0.0)
                for t in range(n_tiles):
                    nc.sync.dma_start(out=OUT[t * P:(t + 1) * P, :], in_=zres[:])
            else:
                edge_phase(table2, 2, emit2)

    nc.compile()
    return nc


# ----------------------------------------------------------------------------
# Host entry point
# ----------------------------------------------------------------------------
def _make_inputs(inputs, meta, idx16, node_of_pos):
    N = meta["N"]; Npad = meta["Npad"]; n_cores = meta["n_cores"]
    x = np.asarray(inputs["x"], dtype=np.float32)
    xp = np.zeros((Npad, F_IN), dtype=np.float32)
    valid = node_of_pos >= 0
    xp[valid] = x[node_of_pos[valid]]
    xT = np.ascontiguousarray(xp.T).astype(bf16)

    W1 = np.asarray(inputs["W1"], dtype=np.float32)
    a_s1 = np.asarray(inputs["a_src1"], dtype=np.float32)
    a_d1 = np.asarray(inputs["a_dst1"], dtype=np.float32)
    A1 = np.zeros((64, 16), dtype=np.float32)
    for h in range(H1):
        A1[h * C1:(h + 1) * C1, h] = a_s1[h]
        A1[h * C1:(h + 1) * C1, 8 + h] = a_d1[h]
    W2 = np.asarray(inputs["W2"], dtype=np.float32)
    a_s2 = np.asarray(inputs["a_src2"], dtype=np.float32).reshape(C2, 1)
    a_d2 = np.asarray(inputs["a_dst2"], dtype=np.float32).reshape(C2, 1)
    A2 = np.concatenate([a_s2, a_d2], axis=1)
    common = dict(
        xT=xT, W1=W1, W1T=np.ascontiguousarray(W1.T), A1=A1,
        W2=W2, W2T=np.ascontiguousarray(W2.T), A2=A2,
        B1=np.asarray(inputs["b1"], np.float32).reshape(1, 64),
        B2=np.asarray(inputs["b2"], np.float32).reshape(1, C2),
    )
    return [dict(common, IDX=np.ascontiguousarray(idx16[k])) for k in range(n_cores)]


def kernel(**inputs):
    x = np.asarray(inputs["x"])
    edge_index = np.asarray(inputs["edge_index"])
    N = x.shape[0]
    meta, idx16, node_of_pos = host_prep(edge_index, N, NCORES)
    nc = build_kernel(meta)
    in_maps = _make_inputs(inputs, meta, idx16, node_of_pos)
    res = run_bass_kernel_spmd(nc, in_maps, list(range(NCORES)))
    out = np.empty((N, C2), dtype=np.float32)
    for k in range(NCORES):
        o = res.results[k]["OUT"]
        pos0 = k * meta["per_core"]
        nodes = node_of_pos[pos0:pos0 + meta["per_core"]]
        valid = nodes >= 0
        out[nodes[valid]] = o[valid.nonzero()[0]]
    return out


# revision 27
# speedup vs baseline: 1.3162x; 1.3162x over previous
"""Self-contained Trainium2 Bass kernel for the 2-layer GAT (nn_GAT_6451040878848).

Sharding: nodes are permuted by quantized in-degree and dealt round-robin to 8
cores; core k owns a contiguous 12800-row octant of the permuted node table
(tile 64 of each octant is a reserved all-pad tile) and aggregates every edge
whose dst is in its octant (dst-octant edge sharding -> no all-reduce). The
only collective is one bf16 AllGather of the layer-2 node table.

Edge phase: per 128-dst tile, edges are slot-major (slot c = c-th in-edge of
each dst; dst == partition), gathered from the node table with dma_gather
(int16 idx). The table is addressed through THREE overlapping 65536-row bank
windows (A=[0,64K), B=[18432,83968), C=[36864,102400)); edges whose src row
falls in an overlap are routed to whichever bank balances the per-dst slot
counts (cuts pad slots ~29% vs a 2-bank split). Pad slots point at zero rows
(alpha_s=-40 => w~e^-8) inside bank-addressable pad tiles, so every index is
non-negative. Weights w = exp(leaky_relu(alpha_s[src]+alpha_d[dst])) (no
max-subtraction: logits are O(1)); the scatter is an identity-lhsT matmul
accumulating 4 slots per Matmult into a [128, 4*vw] PSUM tile, folded with one
tensor_reduce; the softmax denominator rides along as extra rhs columns.
alpha_d never round-trips through DRAM: layer-1 alpha_d is copied to SBUF
during the (replicated) node phase, layer-2 alpha_d is written to SBUF by the
octant-local layer-2 node phase.
"""
import numpy as np
import ml_dtypes

import concourse.bacc as bacc
import concourse.bass as bass
import concourse.tile as tile
from concourse import mybir
from concourse.bass_utils import run_bass_kernel_spmd

P = 128
TROW = 128           # bf16 elements per node-table row (256B)
NEG_SLOPE = 0.2
F_IN = 128
H1, C1 = 8, 8
C2 = 64
DEG_QUANT = 4
NCORES = 8
BANK = 65536
MMG = 4              # slots per matmul group

# ---- fixed geometry (N = 100000 hardcoded) ----
N_NODES = 100000
PER_CORE = 12800             # 100 tiles; tile PAD_TILE is all-pad
N_TILES = PER_CORE // P      # 100
PAD_TILE = 64
J_REAL = 12544               # dealt node slots per core (before pad-tile insertion)
NPAD = PER_CORE * NCORES     # 102400 == rows_total
BASE_A = 32768
B_START = 18432
C_START = NPAD - BANK        # 36864
BASE_B = B_START + 32768     # 51200
BASE_C = C_START + 32768     # 69632
ROW_PAD_A = 4 * PER_CORE + PAD_TILE * P   # 59392 (octant 4 pad tile)
ROW_PAD_C = 5 * PER_CORE + PAD_TILE * P   # 72192 (octant 5 pad tile)
ROW_PAD_B = ROW_PAD_A                      # also in bank B's positive window

bf16 = ml_dtypes.bfloat16


# ----------------------------------------------------------------------------
# Host-side graph preprocessing (integer/index work only)
# ----------------------------------------------------------------------------
def host_prep(edge_index: np.ndarray, n_nodes: int, n_cores: int = NCORES):
    assert n_nodes == N_NODES and n_cores == NCORES
    N = n_nodes
    loops = np.arange(N, dtype=np.int64)
    src = np.concatenate([edge_index[0].astype(np.int64), loops])
    dst = np.concatenate([edge_index[1].astype(np.int64), loops])

    deg = np.bincount(dst, minlength=N)
    degq = -(-deg // DEG_QUANT) * DEG_QUANT

    rank = np.argsort(degq, kind="stable")
    node_rank = np.empty(N, np.int64)
    node_rank[rank] = np.arange(N)
    core_of = node_rank % n_cores
    j_of = node_rank // n_cores              # < 12500 <= J_REAL
    l_of = j_of + np.where(j_of >= PAD_TILE * P, P, 0)
    pos = core_of * PER_CORE + l_of          # == table row
    node_of_pos = np.full(NPAD, -1, dtype=np.int64)
    node_of_pos[pos] = np.arange(N)

    e_core = core_of[dst]
    e_tile = l_of[dst] // P
    e_part = l_of[dst] % P
    r = pos[src]

    inA = r < BANK
    inB = (r >= B_START) & (r < B_START + BANK)
    inC = r >= C_START
    # class 0: A only, 1: A|B, 2: A|B|C, 3: B|C, 4: C only
    cls = np.where(inA & ~inB, 0,
          np.where(inA & inB & ~inC, 1,
          np.where(inA & inB & inC, 2,
          np.where(inB & ~inA, 3, 4))))

    cnt = np.zeros((5, n_cores, N_TILES, P), np.int32)
    for c in range(5):
        m = cls == c
        np.add.at(cnt[c], (e_core[m], e_tile[m], e_part[m]), 1)

    # ---- per-tile (DA, DB, DC) optimization ----
    DA = np.zeros(N_TILES, np.int64)
    DB = np.zeros(N_TILES, np.int64)
    DC = np.zeros(N_TILES, np.int64)
    SPAN = 14
    for t in range(N_TILES):
        n1 = cnt[0, :, t, :].ravel().astype(np.int64)
        n2 = cnt[1, :, t, :].ravel().astype(np.int64)
        n3 = cnt[2, :, t, :].ravel().astype(np.int64)
        n4 = cnt[3, :, t, :].ravel().astype(np.int64)
        n5 = cnt[4, :, t, :].ravel().astype(np.int64)
        if (n1 + n2 + n3 + n4 + n5).max() == 0:
            continue
        da0 = int(n1.max()); dc0 = int(n5.max())
        das = np.arange(da0, da0 + SPAN)[:, None, None]       # [SPAN,1,1]
        dcs = np.arange(dc0, dc0 + SPAN)[None, :, None]       # [1,SPAN,1]
        spareA = das - n1[None, None, :]
        spareC = dcs - n5[None, None, :]
        ov2 = np.maximum(0, n2[None, None, :] - spareA)
        ov4 = np.maximum(0, n4[None, None, :] - spareC)
        sA2 = np.maximum(0, spareA - n2[None, None, :])
        sC2 = np.maximum(0, spareC - n4[None, None, :])
        ov3 = np.maximum(0, n3[None, None, :] - sA2 - sC2)
        db = (ov2 + ov4 + ov3).max(axis=2)                    # [SPAN,SPAN]
        tot = das[:, :, 0] + dcs[:, :, 0] + db
        i, j = np.unravel_index(np.argmin(tot), tot.shape)
        DA[t] = da0 + i; DC[t] = dc0 + j; DB[t] = int(db[i, j])

    # ---- per-dst routing (greedy, honoring (DA, DB, DC)) ----
    # bank_of_edge: 0=A, 1=B, 2=C
    bank = np.full(len(src), -1, dtype=np.int8)
    dstkey = (e_core * N_TILES + e_tile) * P + e_part
    order = np.lexsort((cls, dstkey))
    sk = dstkey[order]; sc = cls[order]
    grp_start = np.concatenate([[0], np.nonzero(np.diff(sk))[0] + 1, [len(sk)]])
    ebank = np.empty(len(order), np.int8)
    slot = np.empty(len(order), np.int64)
    tile_of_key = (np.arange(n_cores * N_TILES * P) // P) % N_TILES
    for gi in range(len(grp_start) - 1):
        s0, s1 = grp_start[gi], grp_start[gi + 1]
        key = sk[s0]; t = tile_of_key[key]
        da, db, dc = int(DA[t]), int(DB[t]), int(DC[t])
        cl = sc[s0:s1]
        c0 = int((cl == 0).sum()); c1 = int((cl == 1).sum())
        c2 = int((cl == 2).sum()); c3 = int((cl == 3).sum())
        c4 = int((cl == 4).sum())
        aspare = da - c0; cspare = dc - c4
        q1A = min(c1, aspare)
        q3C = min(c3, cspare)
        aspare2 = aspare - q1A; cspare2 = cspare - q3C
        q2A = min(c2, aspare2)
        q2C = min(c2 - q2A, cspare2)
        nB = (c1 - q1A) + (c3 - q3C) + (c2 - q2A - q2C)
        assert aspare >= 0 and cspare >= 0 and nB <= db, (t, da, db, dc)
        bk = np.empty(s1 - s0, np.int8)
        u1 = u2 = u3 = 0
        for i, c in enumerate(cl):
            if c == 0:
                bk[i] = 0
            elif c == 4:
                bk[i] = 2
            elif c == 1:
                bk[i] = 0 if u1 < q1A else 1
                u1 += 1
            elif c == 3:
                bk[i] = 2 if u3 < q3C else 1
                u3 += 1
            else:
                bk[i] = 0 if u2 < q2A else (2 if u2 < q2A + q2C else 1)
                u2 += 1
        # segment-relative slots, in class order of appearance
        sl = np.empty(s1 - s0, np.int64)
        ca = cb = cc = 0
        for i in range(s1 - s0):
            if bk[i] == 0:
                sl[i] = ca; ca += 1
            elif bk[i] == 1:
                sl[i] = cb; cb += 1
            else:
                sl[i] = cc; cc += 1
        ebank[s0:s1] = bk
        slot[s0:s1] = sl

    # ---- trailing-negative terminators ----
    # The ucode drops a contiguous all-negative tail of each gather's idx
    # list, so the LAST list position (slot d-1, part 127) of each bank
    # segment must be non-negative. If partition 127's segment is full of
    # negative-idx reals for some core, widen that bank by one all-pad slot.
    bases = np.array([BASE_A, BASE_B, BASE_C], np.int64)
    padrow = np.array([ROW_PAD_A, ROW_PAD_B, ROW_PAD_C], np.int64)
    so_core = e_core[order]; so_row = r[order]
    so_tile = e_tile[order]; so_part = e_part[order]
    so_bank = ebank; so_slot = slot
    Ds = np.stack([DA, DB, DC], axis=1)           # [T, 3]
    m127 = so_part == 127
    for t in range(N_TILES):
        for b in range(3):
            d = int(Ds[t, b])
            if d == 0:
                continue
            m = m127 & (so_tile == t) & (so_bank == b)
            if not m.any():
                continue
            biased = so_row[m] - bases[b]
            cores = so_core[m]
            for k in np.unique(cores):
                mk = cores == k
                if mk.sum() == d and (biased[mk] < 0).all():
                    Ds[t, b] = d + 1
                    break
    DA, DB, DC = Ds[:, 0], Ds[:, 1], Ds[:, 2]

    # ---- idx16 arrays (block-bank-major: per 4-tile block [A t0..t3|B ...|C ...]) ----
    TB = 4
    n_blocks = N_TILES // TB
    Dsum = DA + DB + DC
    # per-tile slot tables (bank-segment layout [A|B|C] per tile)
    tabs = []
    for t in range(N_TILES):
        ds = int(Dsum[t])
        if ds == 0:
            tabs.append(None)
            continue
        tab = np.empty((n_cores, ds, P), np.int64)
        for b in range(3):
            lo = (0, DA[t], DA[t] + DB[t])[b]
            d = (DA[t], DB[t], DC[t])[b]
            if d:
                tab[:, lo:lo + d, :] = padrow[b] - bases[b]
        m = so_tile == t
        seg_lo = np.array([0, DA[t], DA[t] + DB[t]], np.int64)
        abs_slot = seg_lo[so_bank[m]] + so_slot[m]
        biased = so_row[m] - bases[so_bank[m]]
        tab[so_core[m], abs_slot, so_part[m]] = biased
        assert tab.min() >= -32768 and tab.max() <= 32767
        for b in range(3):
            lo = (0, DA[t], DA[t] + DB[t])[b]
            d = int((DA[t], DB[t], DC[t])[b])
            if d == 0:
                continue
            for k in range(n_cores):
                col = tab[k, lo:lo + d, 127]
                if col[d - 1] < 0:
                    nn = np.nonzero(col >= 0)[0]
                    assert len(nn), (t, b, k, "all-negative p127 segment")
                    j = nn[-1]
                    col[d - 1], col[j] = col[j], col[d - 1]
        tabs.append(tab)

    idx_cols = int(Dsum.sum()) * 8
    idx16 = np.zeros((n_cores, 16, max(idx_cols, 1)), np.int16)
    blk_off = []           # column offset of each block
    cols = 0
    for blk in range(n_blocks):
        blk_off.append(cols)
        for b in range(3):
            for ti in range(TB):
                t = blk * TB + ti
                lo = (0, DA[t], DA[t] + DB[t])[b]
                d = int((DA[t], DB[t], DC[t])[b])
                if d == 0 or tabs[t] is None:
                    continue
                seg = tabs[t][:, lo:lo + d, :]
                idx16[:, :, cols:cols + d * 8] = (
                    seg.reshape(n_cores, -1, 16).transpose(0, 2, 1).astype(np.int16))
                cols += d * 8
    assert cols == idx_cols
    idx16 = np.tile(idx16, (1, 8, 1))

    meta = dict(
        N=N, Npad=NPAD, n_cores=n_cores, per_core=PER_CORE, n_tiles=N_TILES,
        DA=[int(v) for v in DA], DB=[int(v) for v in DB], DC=[int(v) for v in DC],
        TB=TB, blk_off=blk_off, idx_cols=int(max(idx_cols, 1)),
    )
    return meta, idx16, node_of_pos


# ----------------------------------------------------------------------------
# Device kernel
# ----------------------------------------------------------------------------
def build_kernel(meta):
    import os
    Npad = meta["Npad"]; n_cores = meta["n_cores"]; per_core = meta["per_core"]
    n_tiles = meta["n_tiles"]
    DA, DB, DC = meta["DA"], meta["DB"], meta["DC"]
    idx_cols = meta["idx_cols"]
    n_groups_per_oct = per_core // (4 * P)        # 25

    TB = meta["TB"]; blk_off = meta["blk_off"]
    n_blocks = n_tiles // TB
    nc = bacc.Bacc("TRN2", target_bir_lowering=False, debug=False,
                   num_devices=n_cores, num_swdge_queues=4)
    f32, b16, i16 = mybir.dt.float32, mybir.dt.bfloat16, mybir.dt.int16
    AF = mybir.ActivationFunctionType
    OP = mybir.AluOpType

    xT = nc.dram_tensor("xT", [F_IN, Npad], b16, kind="ExternalInput").ap()
    W1 = nc.dram_tensor("W1", [F_IN, 64], f32, kind="ExternalInput").ap()
    W1T = nc.dram_tensor("W1T", [64, F_IN], f32, kind="ExternalInput").ap()
    A1 = nc.dram_tensor("A1", [64, 16], f32, kind="ExternalInput").ap()
    W2 = nc.dram_tensor("W2", [64, C2], f32, kind="ExternalInput").ap()
    W2T = nc.dram_tensor("W2T", [C2, 64], f32, kind="ExternalInput").ap()
    A2 = nc.dram_tensor("A2", [C2, 2], f32, kind="ExternalInput").ap()
    B1 = nc.dram_tensor("B1", [1, 64], f32, kind="ExternalInput").ap()
    B2 = nc.dram_tensor("B2", [1, C2], f32, kind="ExternalInput").ap()
    IDX = nc.dram_tensor("IDX", [P, idx_cols], i16, kind="ExternalInput").ap()
    OUT = nc.dram_tensor("OUT", [per_core, C2], f32, kind="ExternalOutput").ap()

    with tile.TileContext(nc) as tc:
        with tc.tile_pool(name="dram", bufs=1, space="DRAM") as dram, \
             tc.tile_pool(name="consts", bufs=1) as cp, \
             tc.tile_pool(name="stg", bufs=3) as nsp, \
             tc.tile_pool(name="xtp", bufs=3) as xtp, \
             tc.tile_pool(name="gpl", bufs=2) as gp, \
             tc.tile_pool(name="vwp", bufs=3) as vwp, \
             tc.tile_pool(name="stat", bufs=4) as sp, \
             tc.tile_pool(name="pacc", bufs=2, space="PSUM") as pacc, \
             tc.tile_pool(name="pnode", bufs=2, space="PSUM") as pnode, \
             tc.tile_pool(name="ptr", bufs=2, space="PSUM") as ptr, \
             tc.tile_pool(name="pl2", bufs=2, space="PSUM") as pl2:

            table1 = dram.tile([Npad, TROW], b16)
            table2 = dram.tile([Npad, TROW], b16)
            h2loc = dram.tile([per_core, TROW], b16)

            # ------------- constants -------------
            ident = cp.tile([P, P], b16)
            nc.gpsimd.memset(ident[:], 0.0)
            iota_i = cp.tile([P, 1], mybir.dt.int32)
            nc.gpsimd.iota(iota_i[:], pattern=[[0, 1]], base=0, channel_multiplier=1)
            iota_f = cp.tile([P, 1], f32)
            nc.vector.tensor_copy(out=iota_f[:], in_=iota_i[:])
            iotar_i = cp.tile([P, P], mybir.dt.int32)
            nc.gpsimd.iota(iotar_i[:], pattern=[[1, P]], base=0, channel_multiplier=0)
            iotar_f = cp.tile([P, P], f32)
            nc.vector.tensor_copy(out=iotar_f[:], in_=iotar_i[:])
            nc.vector.tensor_scalar(out=ident[:], in0=iotar_f[:], scalar1=iota_f[:],
                                    scalar2=None, op0=OP.is_equal)

            w1f = cp.tile([P, 64], f32)
            nc.sync.dma_start(out=w1f[:], in_=W1)
            w1t = cp.tile([64, P], f32)
            nc.sync.dma_start(out=w1t[:], in_=W1T)
            a1t = cp.tile([64, 16], f32)
            nc.sync.dma_start(out=a1t[:], in_=A1)
            w2f = cp.tile([64, 64], f32)
            nc.sync.dma_start(out=w2f[:], in_=W2)
            w2t = cp.tile([64, 64], f32)
            nc.sync.dma_start(out=w2t[:], in_=W2T)
            a2t = cp.tile([64, 2], f32)
            nc.sync.dma_start(out=a2t[:], in_=A2)
            b1r = cp.tile([1, 64], f32)
            nc.sync.dma_start(out=b1r[:], in_=B1)
            b1b = cp.tile([P, 64], f32)
            nc.gpsimd.partition_broadcast(b1b[:], b1r[:])
            b2r = cp.tile([1, 64], f32)
            nc.sync.dma_start(out=b2r[:], in_=B2)
            b2b = cp.tile([P, 64], f32)
            nc.gpsimd.partition_broadcast(b2b[:], b2r[:])

            wext1 = cp.tile([P, 80], b16)
            ws_ps = pnode.tile([P, 352], f32, space="PSUM", tag="np")
            nc.tensor.matmul(out=ws_ps[:, 0:16], lhsT=w1t[:], rhs=a1t[:], start=True, stop=True)
            nc.vector.tensor_copy(out=wext1[:, 0:64], in_=w1f[:])
            nc.vector.tensor_copy(out=wext1[:, 64:80], in_=ws_ps[:, 0:16])

            w2ext = cp.tile([64, 66], b16)
            ws2_ps = pnode.tile([P, 352], f32, space="PSUM", tag="np")
            nc.tensor.matmul(out=ws2_ps[:64, 0:2], lhsT=w2t[:], rhs=a2t[:], start=True, stop=True)
            nc.vector.tensor_copy(out=w2ext[:, 0:64], in_=w2f[:])
            nc.vector.tensor_copy(out=w2ext[:, 64:66], in_=ws2_ps[:64, 0:2])

            # pad-row template: h = 0, alpha = -40
            padt = cp.tile([P, 16], b16)
            nc.gpsimd.memset(padt[:], -40.0)
            padt2 = cp.tile([P, 66], b16)
            nc.gpsimd.memset(padt2[:], 0.0)
            nc.gpsimd.memset(padt2[:, 64:66], -40.0)

            # alpha_d SBUF tables
            ad1_all = cp.tile([P, n_cores * n_tiles * 8], b16)
            ad2_all = cp.tile([P, n_tiles], f32)

            pid = nc.partition_id()

            # ------------- L1 node phase (replicated) -------------
            # Packed layout: matmul i uses lhsT = xt[:, i::4] so output
            # partition p' holds node 4p'+i; the 512-row table store is then
            # per-partition contiguous (128 descs x 1KB instead of 512 x 160B).
            for _nrep in range(int(os.environ.get("KREP_NODE", "1"))):
              for k in range(n_cores):
                for g in range(n_groups_per_oct):
                    base = k * per_core + g * 4 * P
                    xt = xtp.tile([P, 4 * P], b16, tag="xt")
                    nc.sync.dma_start(out=xt[:], in_=xT[:, base:base + 4 * P])
                    xtv = xt[:].rearrange("f (s i) -> f s i", i=4)
                    ps = pnode.tile([P, 352], f32, space="PSUM", tag="np")
                    stage = nsp.tile([P, 4 * TROW], b16, tag="stage")
                    for i in range(4):
                        nc.tensor.matmul(out=ps[:, i * 72:(i + 1) * 72],
                                         lhsT=xtv[:, :, i],
                                         rhs=wext1[:, 0:72], start=True, stop=True)
                    # alpha_d (node-per-partition) in cols 288:320
                    for s in range(4):
                        nc.tensor.matmul(out=ps[:, 288 + s * 8:288 + (s + 1) * 8],
                                         lhsT=xt[:, s * P:(s + 1) * P],
                                         rhs=wext1[:, 72:80], start=True, stop=True)
                    nc.scalar.activation(
                        out=stage[:].rearrange("p (i r) -> p i r", r=TROW)[:, :, 0:72],
                        in_=ps[:, 0:288].rearrange("p (i r) -> p i r", r=72),
                        func=AF.Copy)
                    nc.sync.dma_start(
                        out=table1[base:base + 4 * P, :].rearrange("(p i) r -> p (i r)", p=P),
                        in_=stage[:])
                    nc.vector.tensor_copy(
                        out=ad1_all[:, (k * n_tiles + g * 4) * 8:(k * n_tiles + g * 4 + 4) * 8],
                        in_=ps[:, 288:320])
            # patch pad tiles: alpha_s (cols 64:72) = -40  (h already 0)
            for k in range(n_cores):
                r0 = k * per_core + PAD_TILE * P
                nc.sync.dma_start(
                    out=table1[r0:r0 + P, 64:72].rearrange("(o p) r -> p (o r)", p=P),
                    in_=padt[:, 0:8])

            # ------------- edge phase -------------
            # block geometry: per 4-tile block, bank-major concatenated segments
            blocks = []
            for blk in range(n_blocks):
                segs = []          # (bank, t, blk_slot_off, d)
                calls = []         # (bank, call_slot_off, call_len)
                goff = 0
                for b in range(3):
                    call_lo = goff
                    for ti in range(TB):
                        t = blk * TB + ti
                        d = (DA[t], DB[t], DC[t])[b]
                        if d:
                            segs.append((b, t, goff, d))
                            goff += d
                    if goff > call_lo:
                        calls.append((b, call_lo, goff - call_lo))
                blocks.append((segs, calls, goff))
            DsblkMax = max(g for _, _, g in blocks)

            idxall = cp.tile([P, idx_cols], i16)
            nc.sync.dma_start(out=idxall[:], in_=IDX)

            g_shared = [None]

            def edge_phase(table, layer, emit):
                nogather = os.environ.get("E_NOGATHER") == "1"
                nocompute = os.environ.get("E_NOCOMPUTE") == "1"
                if nogather and g_shared[0] is None:
                    gsh = cp.tile([P, DsblkMax * TROW], b16)
                    nc.gpsimd.memset(gsh[:], 0.125)
                    g_shared[0] = gsh
                heads = H1 if layer == 1 else 1
                vw = 72 if layer == 1 else 65
                in_ap = (table[BASE_A:BASE_A + 32768, :],
                         table[BASE_B:BASE_B + 32768, :],
                         table[BASE_C:min(BASE_C + 32768, Npad), :])
                qctr = 0
                for blk in range(n_blocks):
                    segs, calls, dsblk = blocks[blk]
                    if dsblk == 0:
                        continue
                    G = g_shared[0] if nogather else gp.tile([P, dsblk * TROW], b16, tag="G")
                    if not nogather:
                        for b, call_lo, dlen in calls:
                            c0 = blk_off[blk] + call_lo * 8
                            nc.gpsimd.dma_gather(
                                out_ap=G[:, call_lo * TROW:(call_lo + dlen) * TROW]
                                    .rearrange("p (s r) -> p s r", r=TROW),
                                in_ap=in_ap[b], idxs_ap=idxall[:, c0:c0 + dlen * 8],
                                num_idxs=dlen * P, num_idxs_reg=dlen * P, elem_size=TROW,
                                queue_num=qctr % 4, single_packet=False)
                            qctr += 1
                    if nocompute:
                        continue

                    Gv = G[:, 0:dsblk * TROW].rearrange("p (s r) -> p s r", r=TROW)
                    if layer == 1:
                        ad_off = pid * (n_tiles * 8) + blk * TB * 8
                        ad_blk = sp.tile([P, TB * 8], b16, tag="adt")
                        nc.vector.tensor_copy(out=ad_blk[:], in_=ad1_all[:, bass.ds(ad_off, TB * 8)])
                    w_all = sp.tile([P, dsblk * heads], f32, tag="wf")
                    wb_all = sp.tile([P, dsblk * heads], b16, tag="wb")
                    e_t = sp.tile([P, dsblk * heads], f32, tag="et")
                    if layer == 1:
                        for b, t, off, d in segs:
                            ti = t - blk * TB
                            adv = ad_blk[:, ti * 8:(ti + 1) * 8] \
                                .unsqueeze(1).broadcast_to([P, d, 8])
                            nc.vector.tensor_tensor(
                                out=e_t[:, off * 8:(off + d) * 8]
                                    .rearrange("p (s h) -> p s h", h=8),
                                in0=Gv[:, off:off + d, 64:72], in1=adv, op=OP.add)
                    else:
                        for b, t, off, d in segs:
                            nc.vector.tensor_scalar(
                                out=e_t[:, off:off + d],
                                in0=Gv[:, off:off + d, 64:65]
                                    .rearrange("p s one -> p (s one)"),
                                scalar1=ad2_all[:, t:t + 1], scalar2=None, op0=OP.add)
                    nc.scalar.activation(out=e_t[:], in_=e_t[:],
                                         func=AF.Prelu, alpha=NEG_SLOPE)
                    nc.scalar.activation(out=w_all[:], in_=e_t[:], func=AF.Exp)
                    nc.vector.tensor_copy(out=wb_all[:], in_=w_all[:])

                    # per-tile V staging + scatter matmuls
                    for ti in range(TB):
                        t = blk * TB + ti
                        ds = DA[t] + DB[t] + DC[t]
                        if ds == 0:
                            continue
                        tsegs = [(b, off, d) for (b, t2, off, d) in segs if t2 == t]
                        nmm = -(-ds // MMG)
                        ds_pad = nmm * MMG
                        Vw = vwp.tile([P, ds_pad * vw], b16, tag="vw")
                        if ds_pad > ds:
                            nc.gpsimd.memset(Vw[:, ds * vw:ds_pad * vw], 0.0)
                        Vv = Vw[:].rearrange("p (s c) -> p s c", c=vw)[:, 0:ds, :]
                        tloc = 0
                        for b, off, d in tsegs:
                            if layer == 1:
                                wbv = wb_all[:].rearrange("p (s h) -> p s h", h=8) \
                                    [:, off:off + d, :].unsqueeze(3) \
                                    .broadcast_to([P, d, 8, 8])
                                nc.vector.tensor_tensor(
                                    out=Vv[:, tloc:tloc + d, 0:64]
                                        .rearrange("p s (h c) -> p s h c", c=8),
                                    in0=Gv[:, off:off + d, 0:64]
                                        .rearrange("p s (h c) -> p s h c", c=8),
                                    in1=wbv, op=OP.mult)
                                nc.vector.tensor_copy(
                                    out=Vv[:, tloc:tloc + d, 64:72],
                                    in_=wb_all[:].rearrange("p (s h) -> p s h", h=8)
                                        [:, off:off + d, :])
                            else:
                                wbv = wb_all[:, off:off + d].unsqueeze(2) \
                                    .broadcast_to([P, d, 64])
                                nc.vector.tensor_tensor(
                                    out=Vv[:, tloc:tloc + d, 0:64],
                                    in0=Gv[:, off:off + d, 0:64], in1=wbv, op=OP.mult)
                                nc.vector.tensor_copy(
                                    out=Vv[:, tloc:tloc + d, 64:65],
                                    in_=wb_all[:, off:off + d].unsqueeze(2))
                            tloc += d

                        acc = pacc.tile([P, MMG * 72], f32, space="PSUM", tag="acc")
                        for i in range(nmm):
                            nc.tensor.matmul(out=acc[:, 0:MMG * vw], lhsT=ident[:],
                                             rhs=Vw[:, i * MMG * vw:(i + 1) * MMG * vw],
                                             start=(i == 0), stop=(i == nmm - 1))
                        emit(t, acc, MMG, vw)

            # ------------- L1 -------------
            out1 = cp.tile([P, n_tiles * 72], b16)

            def emit1(t, acc, ngrp, vw):
                with nc.allow_low_precision(reason="4-way fold to bf16 staging"):
                    nc.vector.tensor_reduce(
                        out=out1[:, t * 72:(t + 1) * 72],
                        in_=acc[:, 0:ngrp * vw].rearrange("p (g c) -> p c g", c=vw),
                        op=OP.add, axis=mybir.AxisListType.X)

            if os.environ.get("SKIP_E1") == "1":
                nc.gpsimd.memset(out1[:], 1.0)
            else:
                for _erep in range(int(os.environ.get("KREP_E1", "1"))):
                    edge_phase(table1, 1, emit1)
                if os.environ.get("E_NOCOMPUTE") == "1":
                    nc.gpsimd.memset(out1[:], 1.0)

            # ------------- L2 node phase (octant-local) -------------
            for t in range(n_tiles):
                if t == PAD_TILE:
                    nc.sync.dma_start(out=h2loc[t * P:(t + 1) * P, 0:66], in_=padt2[:])
                    continue
                if DA[t] + DB[t] + DC[t] == 0:
                    continue
                den = sp.tile([P, 8], f32, tag="den")
                nc.vector.tensor_copy(out=den[:], in_=out1[:, t * 72 + 64:t * 72 + 72])
                rec = sp.tile([P, 8], f32, tag="rec")
                nc.vector.reciprocal(rec[:], den[:])
                recb = sp.tile([P, 8], b16, tag="recb")
                nc.vector.tensor_copy(out=recb[:], in_=rec[:])
                h1f = sp.tile([P, 64], f32, tag="h1f")
                nc.vector.tensor_tensor(
                    out=h1f[:].rearrange("p (h c) -> p h c", c=8),
                    in0=out1[:, t * 72:t * 72 + 64].rearrange("p (h c) -> p h c", c=8),
                    in1=recb[:].unsqueeze(2).broadcast_to([P, 8, 8]), op=OP.mult)
                nc.vector.tensor_tensor(out=h1f[:], in0=h1f[:], in1=b1b[:], op=OP.add)
                # ELU: out = max(x,0) + exp(min(x,0)) - 1
                xm = sp.tile([P, 64], f32, tag="xm")
                nc.vector.tensor_scalar(out=xm[:], in0=h1f[:], scalar1=0.0,
                                        scalar2=None, op0=OP.min)
                xe = sp.tile([P, 64], f32, tag="xe")
                nc.scalar.activation(out=xe[:], in_=xm[:], func=AF.Exp)
                xp = sp.tile([P, 64], b16, tag="xp")
                nc.vector.tensor_scalar(out=xp[:], in0=h1f[:], scalar1=0.0,
                                        scalar2=None, op0=OP.max)
                h1e = sp.tile([P, 64], b16, tag="h1e")
                nc.vector.tensor_scalar(out=h1e[:], in0=xe[:], scalar1=-1.0,
                                        scalar2=None, op0=OP.add, accum_out=None)
                nc.vector.tensor_tensor(out=h1e[:], in0=h1e[:], in1=xp[:], op=OP.add)
                trp = ptr.tile([P, P], b16, space="PSUM", tag="tr")
                nc.tensor.transpose(out=trp[:64, :], in_=h1e[:], identity=ident[:])
                h1t = sp.tile([64, P], b16, tag="h1t")
                nc.scalar.activation(out=h1t[:], in_=trp[:64, :], func=AF.Copy)
                ps2 = pl2.tile([P, 80], f32, space="PSUM", tag="l2")
                nc.tensor.matmul(out=ps2[:, 0:66], lhsT=h1t[:], rhs=w2ext[:],
                                 start=True, stop=True)
                st2 = nsp.tile([P, 65], b16, tag="st2")
                nc.scalar.activation(out=st2[:], in_=ps2[:, 0:65], func=AF.Copy)
                nc.sync.dma_start(out=h2loc[t * P:(t + 1) * P, 0:65], in_=st2[:])
                nc.vector.tensor_copy(out=ad2_all[:, t:t + 1], in_=ps2[:, 65:66])

            # ------------- AllGather h2 octants -> table2 -------------
            if os.environ.get("SKIP_CC") == "1":
                for k in range(n_cores):
                    nc.sync.dma_start(
                        out=table2[k * per_core:(k + 1) * per_core, :],
                        in_=h2loc[:])
            else:
                nc.gpsimd.collective_compute(
                    "AllGather", mybir.AluOpType.bypass,
                    replica_groups=[list(range(n_cores))],
                    ins=[h2loc[:].opt()],
                    outs=[table2[0:Npad, :].opt()],
                )

            # ------------- L2 -------------
            def emit2(t, acc, ngrp, vw):
                o2 = sp.tile([P, 65], f32, tag="o2")
                nc.vector.tensor_reduce(
                    out=o2[:],
                    in_=acc[:, 0:ngrp * vw].rearrange("p (g c) -> p c g", c=vw),
                    op=OP.add, axis=mybir.AxisListType.X)
                rec2 = sp.tile([P, 1], f32, tag="rec2")
                nc.vector.reciprocal(rec2[:], o2[:, 64:65])
                o2n = sp.tile([P, 64], f32, tag="o2n")
                nc.vector.tensor_scalar(out=o2n[:], in0=o2[:, 0:64], scalar1=rec2[:],
                                        scalar2=None, op0=OP.mult)
                nc.vector.tensor_tensor(out=o2n[:], in0=o2n[:], in1=b2b[:], op=OP.add)
                m = sp.tile([P, 1], f32, tag="m")
                nc.vector.tensor_reduce(out=m[:], in_=o2n[:], op=OP.max,
                                        axis=mybir.AxisListType.X)
                negm = sp.tile([P, 1], f32, tag="negm")
                nc.vector.tensor_scalar(out=negm[:], in0=m[:], scalar1=-1.0,
                                        scalar2=None, op0=OP.mult)
                scr = sp.tile([P, 64], f32, tag="scr")
                sume = sp.tile([P, 1], f32, tag="sume")
                nc.scalar.activation(out=scr[:], in_=o2n[:], func=AF.Exp,
                                     bias=negm[:], accum_out=sume[:])
                lns = sp.tile([P, 1], f32, tag="lns")
                nc.scalar.activation(out=lns[:], in_=sume[:], func=AF.Ln)
                res = sp.tile([P, 64], f32, tag="res")
                nc.vector.tensor_scalar(out=res[:], in0=o2n[:], scalar1=m[:],
                                        scalar2=lns[:], op0=OP.subtract,
                                        op1=OP.subtract)
                nc.sync.dma_start(out=OUT[t * P:(t + 1) * P, :], in_=res[:])

            if os.environ.get("SKIP_E2") == "1":
                zres = sp.tile([P, 64], f32, tag="zres")
                nc.gpsimd.memset(zres[:], 0.0)
                for t in range(n_tiles):
                    nc.sync.dma_start(out=OUT[t * P:(t + 1) * P, :], in_=zres[:])
            else:
                for _erep in range(int(os.environ.get("KREP_E2", "1"))):
                    edge_phase(table2, 2, emit2)

    nc.compile()
    return nc


# ----------------------------------------------------------------------------
# Host entry point
# ----------------------------------------------------------------------------
def _make_inputs(inputs, meta, idx16, node_of_pos):
    N = meta["N"]; Npad = meta["Npad"]; n_cores = meta["n_cores"]
    x = np.asarray(inputs["x"], dtype=np.float32)
    xp = np.zeros((Npad, F_IN), dtype=np.float32)
    valid = node_of_pos >= 0
    xp[valid] = x[node_of_pos[valid]]
    xT = np.ascontiguousarray(xp.T).astype(bf16)

    W1 = np.asarray(inputs["W1"], dtype=np.float32)
    a_s1 = np.asarray(inputs["a_src1"], dtype=np.float32)
    a_d1 = np.asarray(inputs["a_dst1"], dtype=np.float32)
    A1 = np.zeros((64, 16), dtype=np.float32)
    for h in range(H1):
        A1[h * C1:(h + 1) * C1, h] = a_s1[h]
        A1[h * C1:(h + 1) * C1, 8 + h] = a_d1[h]
    W2 = np.asarray(inputs["W2"], dtype=np.float32)
    a_s2 = np.asarray(inputs["a_src2"], dtype=np.float32).reshape(C2, 1)
    a_d2 = np.asarray(inputs["a_dst2"], dtype=np.float32).reshape(C2, 1)
    A2 = np.concatenate([a_s2, a_d2], axis=1)
    common = dict(
        xT=xT, W1=W1, W1T=np.ascontiguousarray(W1.T), A1=A1,
        W2=W2, W2T=np.ascontiguousarray(W2.T), A2=A2,
        B1=np.asarray(inputs["b1"], np.float32).reshape(1, 64),
        B2=np.asarray(inputs["b2"], np.float32).reshape(1, C2),
    )
    return [dict(common, IDX=np.ascontiguousarray(idx16[k])) for k in range(n_cores)]


def kernel(**inputs):
    x = np.asarray(inputs["x"])
    edge_index = np.asarray(inputs["edge_index"])
    N = x.shape[0]
    meta, idx16, node_of_pos = host_prep(edge_index, N, NCORES)
    nc = build_kernel(meta)
    in_maps = _make_inputs(inputs, meta, idx16, node_of_pos)
    res = run_bass_kernel_spmd(nc, in_maps, list(range(NCORES)))
    out = np.empty((N, C2), dtype=np.float32)
    for k in range(NCORES):
        o = res.results[k]["OUT"]
        pos0 = k * meta["per_core"]
        nodes = node_of_pos[pos0:pos0 + meta["per_core"]]
        valid = nodes >= 0
        out[nodes[valid]] = o[valid.nonzero()[0]]
    return out
